# revision 2
# baseline (speedup 1.0000x reference)
import sys

sys.path.insert(0, "/opt/trn_rl_repo")

from contextlib import ExitStack

import ml_dtypes
import numpy as np

import concourse.bass as bass
import concourse.mybir as mybir
import concourse.tile as tile
from concourse import bacc, bass_utils

N, OBS, ENC, ACT, K = 16384, 512, 512, 64, 8
ALPHA = 1.0
NCORES = 8
P = 128
NJ = 5  # DoubleRow slab pairs: (x0a,x0b),(x0c,x0d),(x1a,x1b),(x1c,x1d),(u,0)
F32 = mybir.dt.float32
FP8 = mybir.dt.float8e4
DR = mybir.MatmulPerfMode.DoubleRow
NP8 = ml_dtypes.float8_e4m3


def build_nc(t_max):
    # Expert-sharded: core c holds expert c's rows (t_max 128-row tiles, padded
    # with zero rows). Per tile: 5 fp8 DoubleRow matmuls accumulate
    #   e = x0@(4M_c) + u@(8B_c) - x1@(4W^T)   (pairwise scales cancel)
    # into one PSUM bank; ACT squares straight from PSUM with accumulate.
    nc = bacc.Bacc("TRN2", target_bir_lowering=False)
    zt = nc.declare_dram_parameter("zt", [t_max, P, 2 * NJ, P], FP8, isOutput=False)
    dms = [
        nc.declare_dram_parameter(f"d{j}", [P, 2, ENC], FP8, isOutput=False)
        for j in range(NJ)
    ]
    loss = nc.declare_dram_parameter("loss_out", [1, 1], F32, isOutput=True)

    with tile.TileContext(nc) as tc, ExitStack() as ctx:
        const = ctx.enter_context(tc.tile_pool(name="const", bufs=1))
        stream = ctx.enter_context(tc.tile_pool(name="stream", bufs=t_max))
        dwork = ctx.enter_context(tc.tile_pool(name="dwork", bufs=4))
        psum = ctx.enter_context(tc.tile_pool(name="psum", bufs=8, space="PSUM"))

        d_sb = []
        for j, dm in enumerate(dms):
            dj = const.tile([P, 2, ENC], FP8, name=f"dsb{j}")
            nc.sync.dma_start(dj[:], dm[:])
            d_sb.append(dj)
        acc = const.tile([P, t_max], F32)

        zs = []
        for t in range(t_max):
            z = stream.tile([P, 2 * NJ, P], FP8, name="z")
            nc.sync.dma_start(z[:], zt[t])
            zs.append(z)

        for t in range(t_max):
            ps = psum.tile([P, ENC], F32, name="ps")
            for j in range(NJ):
                nc.tensor.matmul(
                    ps[:],
                    zs[t][:, 2 * j : 2 * j + 2, :],
                    d_sb[j][:],
                    start=(j == 0),
                    stop=(j == NJ - 1),
                    perf_mode=DR,
                )
            sj = dwork.tile([P, ENC], mybir.dt.bfloat16, name="sj")
            nc.scalar.activation(
                sj[:],
                ps[:],
                mybir.ActivationFunctionType.Square,
                accum_out=acc[:, t : t + 1],
            )

        out_sb = const.tile([1, 1], F32)
        nc.gpsimd.tensor_reduce(
            out_sb[:], acc[:], axis=mybir.AxisListType.XYZWC, op=mybir.AluOpType.add
        )
        nc.sync.dma_start(loss[:], out_sb[:])

    nc.finalize()
    return nc


_NC_CACHE = {}
_LAST_TMAX = None


def _get_nc(t_max=None):
    t = _LAST_TMAX if t_max is None else t_max
    if t not in _NC_CACHE:
        _NC_CACHE[t] = build_nc(t)
    return _NC_CACHE[t]


def make_in_maps(X1, X0, U, W_enc, A_all, B_rest, C_w, C_b):
    global _LAST_TMAX
    X1, X0, U = np.asarray(X1), np.asarray(X0), np.asarray(U)
    W_enc, A_all, B_rest = np.asarray(W_enc), np.asarray(A_all), np.asarray(B_rest)
    C_w, C_b = np.asarray(C_w), np.asarray(C_b)

    # f64 router on host: argmax(X0 @ W_enc.T @ C_w.T + C_b) per row
    m = (C_w.astype(np.float64) @ W_enc.astype(np.float64)).T  # [OBS, K]
    inds = np.argmax(X0.astype(np.float64) @ m + C_b.astype(np.float64), axis=1)
    counts = np.bincount(inds, minlength=K)
    t_max = max(1, -(-int(counts.max()) // P))
    _LAST_TMAX = t_max

    # quantize data once (pair scales cancel against the matrices)
    x0q = (X0 * 0.25).astype(NP8)
    x1q = (X1 * 0.25).astype(NP8)
    uq = (U * 0.125).astype(NP8)

    wT = W_enc.T.astype(np.float32)  # [OBS, ENC]
    wn4 = -4.0 * wT
    B0 = np.eye(ENC, dtype=np.float32)[:ACT]
    Ball = np.concatenate([B0[None], B_rest.astype(np.float32)], axis=0)

    in_maps = []
    for c in range(K):
        rows = np.nonzero(inds == c)[0]
        nr = len(rows)
        zz = np.zeros((2 * NJ, P, t_max * P), NP8)  # [slab, comp, n]
        zz[0:4, :, :nr] = x0q[rows].T.reshape(4, P, nr)
        zz[4:8, :, :nr] = x1q[rows].T.reshape(4, P, nr)
        zz[8, :ACT, :nr] = uq[rows].T
        z = np.ascontiguousarray(
            zz.reshape(2 * NJ, P, t_max, P).transpose(2, 1, 0, 3)
        )  # [t, p, slab, r]

        m4 = 4.0 * (wT @ A_all[c].T.astype(np.float32))  # [OBS, ENC]
        dslab = np.zeros((2 * NJ, P, ENC), np.float32)
        dslab[0:4] = m4.reshape(4, P, ENC)
        dslab[4:8] = wn4.reshape(4, P, ENC)
        dslab[8, :ACT, :] = 8.0 * Ball[c]
        d8 = dslab.astype(NP8)

        im = {"zt": z}
        for j in range(NJ):
            im[f"d{j}"] = np.ascontiguousarray(d8[2 * j : 2 * j + 2].transpose(1, 0, 2))
        in_maps.append(im)
    return in_maps


def kernel(X1, X0, U, W_enc, A_all, B_rest, C_w, C_b):
    in_maps = make_in_maps(X1, X0, U, W_enc, A_all, B_rest, C_w, C_b)
    nc = _get_nc()
    res = bass_utils.run_bass_kernel_spmd(nc, in_maps, list(range(NCORES)))
    total = sum(float(r["loss_out"][0, 0]) for r in res.results)
    return np.float32(ALPHA * total / (ENC * N))


# revision 4
# speedup vs baseline: 2.2943x; 2.2943x over previous
import sys

sys.path.insert(0, "/opt/trn_rl_repo")

from contextlib import ExitStack

import ml_dtypes
import numpy as np

import concourse.bass as bass
import concourse.mybir as mybir
import concourse.tile as tile
from concourse import bacc, bass_utils

N, OBS, ENC, ACT, K = 16384, 512, 512, 64, 8
ALPHA = 1.0
NCORES = 8
P = 128
NJ = 5  # DoubleRow slab pairs: (x0a,x0b),(x0c,x0d),(x1a,x1b),(x1c,x1d),(u,0)
F32 = mybir.dt.float32
BF16 = mybir.dt.bfloat16
FP8 = mybir.dt.float8e4
DR = mybir.MatmulPerfMode.DoubleRow
NP8 = ml_dtypes.float8_e4m3
NWARM = 12


def build_nc(t_max):
    # Expert-sharded: core c holds expert c's rows (t_max 128-row tiles, padded
    # with zero rows). Per tile: 5 fp8 DoubleRow matmuls accumulate
    #   e = x0@(4M_c) + u@(8B_c) - x1@(4W^T)   (pairwise scales cancel)
    # into one PSUM bank; ACT squares from PSUM, DVE reduces, Pool sums at end.
    nc = bacc.Bacc("TRN2", target_bir_lowering=False)
    zt = nc.declare_dram_parameter("zt", [t_max, P, 2 * NJ, P], FP8, isOutput=False)
    dms = [
        nc.declare_dram_parameter(f"d{j}", [P, 2, ENC], FP8, isOutput=False)
        for j in range(NJ)
    ]
    loss = nc.declare_dram_parameter("loss_out", [1, 1], F32, isOutput=True)

    ng = (t_max + 1) // 2  # z DMA groups of 2 tiles

    with tile.TileContext(nc) as tc, ExitStack() as ctx:
        const = ctx.enter_context(tc.tile_pool(name="const", bufs=1))
        stream = ctx.enter_context(tc.tile_pool(name="stream", bufs=ng))
        dwork = ctx.enter_context(tc.tile_pool(name="dwork", bufs=4))
        psum = ctx.enter_context(tc.tile_pool(name="psum", bufs=7, space="PSUM"))
        psumw = ctx.enter_context(tc.tile_pool(name="psumw", bufs=1, space="PSUM"))

        # PE warmup on uninitialized scratch: ramps the p-state while DMAs run.
        # Results land in a dedicated PSUM bank and are never read.
        wz = const.tile([P, 2, P], FP8)
        wd = const.tile([P, 2, ENC], FP8)
        nc.gpsimd.memset(wz[:], 0)
        nc.gpsimd.memset(wd[:], 0)
        pw = psumw.tile([P, ENC], F32, name="pw")
        for _ in range(NWARM):
            nc.tensor.matmul(pw[:], wz[:], wd[:], start=True, stop=True, perf_mode=DR)

        d_sb = []
        for j, dm in enumerate(dms):
            dj = const.tile([P, 2, ENC], FP8, name=f"dsb{j}")
            nc.sync.dma_start(dj[:], dm[:])
            d_sb.append(dj)
        acc = const.tile([P, t_max], F32)

        zs = []
        for g in range(ng):
            w = min(2, t_max - 2 * g)
            zg = stream.tile([P, 2, 2 * NJ, P], FP8, name="zg")
            nc.sync.dma_start(
                zg[:, 0:w], zt[2 * g : 2 * g + w].rearrange("t p s r -> p t s r")
            )
            zs.append(zg)

        for t in range(t_max):
            zg = zs[t // 2]
            i = t % 2
            ps = psum.tile([P, ENC], F32, name="ps")
            for j in range(NJ):
                nc.tensor.matmul(
                    ps[:],
                    zg[:, i, 2 * j : 2 * j + 2, :],
                    d_sb[j][:],
                    start=(j == 0),
                    stop=(j == NJ - 1),
                    perf_mode=DR,
                )
            sj = dwork.tile([P, ENC], BF16, name="sj")
            nc.scalar.activation(sj[:], ps[:], mybir.ActivationFunctionType.Square)
            nc.vector.tensor_reduce(
                acc[:, t : t + 1], sj[:], axis=mybir.AxisListType.X, op=mybir.AluOpType.add
            )

        out_sb = const.tile([1, 1], F32)
        nc.gpsimd.tensor_reduce(
            out_sb[:], acc[:], axis=mybir.AxisListType.XYZWC, op=mybir.AluOpType.add
        )
        nc.sync.dma_start(loss[:], out_sb[:])

    nc.finalize()
    return nc


_NC_CACHE = {}
_LAST_TMAX = None


def _get_nc(t_max=None):
    t = _LAST_TMAX if t_max is None else t_max
    if t not in _NC_CACHE:
        _NC_CACHE[t] = build_nc(t)
    return _NC_CACHE[t]


def make_in_maps(X1, X0, U, W_enc, A_all, B_rest, C_w, C_b):
    global _LAST_TMAX
    X1, X0, U = np.asarray(X1), np.asarray(X0), np.asarray(U)
    W_enc, A_all, B_rest = np.asarray(W_enc), np.asarray(A_all), np.asarray(B_rest)
    C_w, C_b = np.asarray(C_w), np.asarray(C_b)

    # f64 router on host: argmax(X0 @ W_enc.T @ C_w.T + C_b) per row
    m = (C_w.astype(np.float64) @ W_enc.astype(np.float64)).T  # [OBS, K]
    inds = np.argmax(X0.astype(np.float64) @ m + C_b.astype(np.float64), axis=1)
    counts = np.bincount(inds, minlength=K)
    t_max = max(1, -(-int(counts.max()) // P))
    _LAST_TMAX = t_max

    # quantize data once (pair scales cancel against the matrices)
    x0q = (X0 * 0.25).astype(NP8)
    x1q = (X1 * 0.25).astype(NP8)
    uq = (U * 0.125).astype(NP8)

    wT = W_enc.T.astype(np.float32)  # [OBS, ENC]
    wn4 = -4.0 * wT
    B0 = np.eye(ENC, dtype=np.float32)[:ACT]
    Ball = np.concatenate([B0[None], B_rest.astype(np.float32)], axis=0)

    in_maps = []
    for c in range(K):
        rows = np.nonzero(inds == c)[0]
        nr = len(rows)
        zz = np.zeros((2 * NJ, P, t_max * P), NP8)  # [slab, comp, n]
        zz[0:4, :, :nr] = x0q[rows].T.reshape(4, P, nr)
        zz[4:8, :, :nr] = x1q[rows].T.reshape(4, P, nr)
        zz[8, :ACT, :nr] = uq[rows].T
        z = np.ascontiguousarray(
            zz.reshape(2 * NJ, P, t_max, P).transpose(2, 1, 0, 3)
        )  # [t, p, slab, r]

        m4 = 4.0 * (wT @ A_all[c].T.astype(np.float32))  # [OBS, ENC]
        dslab = np.zeros((2 * NJ, P, ENC), np.float32)
        dslab[0:4] = m4.reshape(4, P, ENC)
        dslab[4:8] = wn4.reshape(4, P, ENC)
        dslab[8, :ACT, :] = 8.0 * Ball[c]
        d8 = dslab.astype(NP8)

        im = {"zt": z}
        for j in range(NJ):
            im[f"d{j}"] = np.ascontiguousarray(d8[2 * j : 2 * j + 2].transpose(1, 0, 2))
        in_maps.append(im)
    return in_maps


def kernel(X1, X0, U, W_enc, A_all, B_rest, C_w, C_b):
    in_maps = make_in_maps(X1, X0, U, W_enc, A_all, B_rest, C_w, C_b)
    nc = _get_nc()
    res = bass_utils.run_bass_kernel_spmd(nc, in_maps, list(range(NCORES)))
    total = sum(float(r["loss_out"][0, 0]) for r in res.results)
    return np.float32(ALPHA * total / (ENC * N))


# revision 8
# speedup vs baseline: 2.3025x; 1.0036x over previous
import sys

sys.path.insert(0, "/opt/trn_rl_repo")

from contextlib import ExitStack

import ml_dtypes
import numpy as np

import concourse.bass as bass
import concourse.mybir as mybir
import concourse.tile as tile
from concourse import bacc, bass_utils

N, OBS, ENC, ACT, K = 16384, 512, 512, 64, 8
ALPHA = 1.0
NCORES = 8
P = 128
NJ = 5  # DoubleRow slab pairs: (x0a,x0b),(x0c,x0d),(x1a,x1b),(x1c,x1d),(u,0)
F32 = mybir.dt.float32
BF16 = mybir.dt.bfloat16
FP8 = mybir.dt.float8e4
DR = mybir.MatmulPerfMode.DoubleRow
NP8 = ml_dtypes.float8_e4m3
NWARM = 12


def _zgroups(t_max):
    # small groups first for an early pipeline start, then wide ones
    widths, off = [], 0
    for w in [2, 2]:
        if off < t_max:
            w = min(w, t_max - off)
            widths.append((off, w))
            off += w
    while off < t_max:
        w = min(4, t_max - off)
        widths.append((off, w))
        off += w
    return widths


def build_nc(t_max):
    # Expert-sharded: core c holds expert c's rows (t_max 128-row tiles, padded
    # with zero rows). Per tile: 5 fp8 DoubleRow matmuls accumulate
    #   e = x0@(4M_c) + u@(8B_c) - x1@(4W^T)   (pairwise scales cancel)
    # into one PSUM bank; ACT squares from PSUM, DVE reduces, Pool sums at end.
    nc = bacc.Bacc("TRN2", target_bir_lowering=False)
    zt = nc.declare_dram_parameter("zt", [P, t_max, 2 * NJ, P], FP8, isOutput=False)
    da = nc.declare_dram_parameter("da", [P, 4, ENC], FP8, isOutput=False)
    db = nc.declare_dram_parameter("db", [P, 6, ENC], FP8, isOutput=False)
    loss = nc.declare_dram_parameter("loss_out", [1, 1], F32, isOutput=True)

    groups = _zgroups(t_max)

    with tile.TileContext(nc) as tc, ExitStack() as ctx:
        const = ctx.enter_context(tc.tile_pool(name="const", bufs=1))
        stream = ctx.enter_context(tc.tile_pool(name="stream", bufs=len(groups)))
        dwork = ctx.enter_context(tc.tile_pool(name="dwork", bufs=4))
        psum = ctx.enter_context(tc.tile_pool(name="psum", bufs=7, space="PSUM"))
        psumw = ctx.enter_context(tc.tile_pool(name="psumw", bufs=1, space="PSUM"))

        # PE warmup on zeroed scratch: ramps the p-state while DMAs run.
        # Results land in a dedicated PSUM bank and are never read.
        wz = const.tile([P, 2, P], FP8)
        wd = const.tile([P, 2, ENC], FP8)
        nc.gpsimd.memset(wz[:], 0)
        nc.gpsimd.memset(wd[:], 0)
        pw = psumw.tile([P, ENC], F32, name="pw")
        for _ in range(NWARM):
            nc.tensor.matmul(pw[:], wz[:], wd[:], start=True, stop=True, perf_mode=DR)

        da_sb = const.tile([P, 4, ENC], FP8)
        db_sb = const.tile([P, 6, ENC], FP8)
        acc = const.tile([P, t_max], F32)

        def d_ap(j):
            return da_sb[:, 2 * j : 2 * j + 2, :] if j < 2 else db_sb[
                :, 2 * (j - 2) : 2 * (j - 2) + 2, :
            ]

        # DMA issue order: da, first z group, db, remaining z groups
        nc.sync.dma_start(da_sb[:], da[:])
        zs = []
        for gi, (off, w) in enumerate(groups):
            zg = stream.tile([P, 4, 2 * NJ, P], FP8, name="zg")
            nc.sync.dma_start(zg[:, 0:w], zt[:, off : off + w])
            zs.append(zg)
            if gi == 0:
                nc.sync.dma_start(db_sb[:], db[:])

        t2g = {}
        for gi, (off, w) in enumerate(groups):
            for t in range(off, off + w):
                t2g[t] = gi
        for t in range(t_max):
            gi = t2g[t]
            i = t - groups[gi][0]
            ps = psum.tile([P, ENC], F32, name="ps")
            for j in range(NJ):
                nc.tensor.matmul(
                    ps[:],
                    zs[gi][:, i, 2 * j : 2 * j + 2, :],
                    d_ap(j),
                    start=(j == 0),
                    stop=(j == NJ - 1),
                    perf_mode=DR,
                )
            sj = dwork.tile([P, ENC], BF16, name="sj")
            nc.scalar.activation(sj[:], ps[:], mybir.ActivationFunctionType.Square)
            nc.vector.tensor_reduce(
                acc[:, t : t + 1], sj[:], axis=mybir.AxisListType.X, op=mybir.AluOpType.add
            )

        out_sb = const.tile([1, 1], F32)
        nc.gpsimd.tensor_reduce(
            out_sb[:], acc[:], axis=mybir.AxisListType.XYZWC, op=mybir.AluOpType.add
        )
        nc.sync.dma_start(loss[:], out_sb[:])

    nc.finalize()
    return nc


_NC_CACHE = {}
_LAST_TMAX = None


def _get_nc(t_max=None):
    t = _LAST_TMAX if t_max is None else t_max
    if t not in _NC_CACHE:
        _NC_CACHE[t] = build_nc(t)
    return _NC_CACHE[t]


def make_in_maps(X1, X0, U, W_enc, A_all, B_rest, C_w, C_b):
    global _LAST_TMAX
    X1, X0, U = np.asarray(X1), np.asarray(X0), np.asarray(U)
    W_enc, A_all, B_rest = np.asarray(W_enc), np.asarray(A_all), np.asarray(B_rest)
    C_w, C_b = np.asarray(C_w), np.asarray(C_b)

    # f64 router on host: argmax(X0 @ W_enc.T @ C_w.T + C_b) per row
    m = (C_w.astype(np.float64) @ W_enc.astype(np.float64)).T  # [OBS, K]
    inds = np.argmax(X0.astype(np.float64) @ m + C_b.astype(np.float64), axis=1)
    counts = np.bincount(inds, minlength=K)
    t_max = max(1, -(-int(counts.max()) // P))
    _LAST_TMAX = t_max

    # quantize data once (pair scales cancel against the matrices)
    x0q = (X0 * 0.25).astype(NP8)
    x1q = (X1 * 0.25).astype(NP8)
    uq = (U * 0.125).astype(NP8)

    wT = W_enc.T.astype(np.float32)  # [OBS, ENC]
    wn4 = -4.0 * wT
    B0 = np.eye(ENC, dtype=np.float32)[:ACT]
    Ball = np.concatenate([B0[None], B_rest.astype(np.float32)], axis=0)

    in_maps = []
    for c in range(K):
        rows = np.nonzero(inds == c)[0]
        nr = len(rows)
        zz = np.zeros((2 * NJ, P, t_max * P), NP8)  # [slab, comp, n]
        zz[0:4, :, :nr] = x0q[rows].T.reshape(4, P, nr)
        zz[4:8, :, :nr] = x1q[rows].T.reshape(4, P, nr)
        zz[8, :ACT, :nr] = uq[rows].T
        z = np.ascontiguousarray(
            zz.reshape(2 * NJ, P, t_max, P).transpose(1, 2, 0, 3)
        )  # [p, t, slab, r]

        m4 = 4.0 * (wT @ A_all[c].T.astype(np.float32))  # [OBS, ENC]
        dslab = np.zeros((2 * NJ, P, ENC), np.float32)
        dslab[0:4] = m4.reshape(4, P, ENC)
        dslab[4:8] = wn4.reshape(4, P, ENC)
        dslab[8, :ACT, :] = 8.0 * Ball[c]
        d8 = dslab.astype(NP8)

        in_maps.append(
            {
                "zt": z,
                "da": np.ascontiguousarray(d8[0:4].transpose(1, 0, 2)),
                "db": np.ascontiguousarray(d8[4:10].transpose(1, 0, 2)),
            }
        )
    return in_maps


def kernel(X1, X0, U, W_enc, A_all, B_rest, C_w, C_b):
    in_maps = make_in_maps(X1, X0, U, W_enc, A_all, B_rest, C_w, C_b)
    nc = _get_nc()
    res = bass_utils.run_bass_kernel_spmd(nc, in_maps, list(range(NCORES)))
    total = sum(float(r["loss_out"][0, 0]) for r in res.results)
    return np.float32(ALPHA * total / (ENC * N))


# revision 13
# speedup vs baseline: 2.3973x; 1.0412x over previous
import sys

sys.path.insert(0, "/opt/trn_rl_repo")

from contextlib import ExitStack

import ml_dtypes
import numpy as np

import concourse.bass as bass
import concourse.mybir as mybir
import concourse.tile as tile
from concourse import bacc, bass_utils

N, OBS, ENC, ACT, K = 16384, 512, 512, 64, 8
ALPHA = 1.0
NCORES = 8
P = 128
NJ = 5  # DoubleRow slab pairs: (x0a,x0b),(x0c,x0d),(x1a,x1b),(x1c,x1d),(u,0)
F32 = mybir.dt.float32
BF16 = mybir.dt.bfloat16
FP8 = mybir.dt.float8e4
DR = mybir.MatmulPerfMode.DoubleRow
NP8 = ml_dtypes.float8_e4m3
NWARM = 10


def _solve_assign(pat, needs):
    # slots: NCORES copies of each nonzero size in pat; find per-expert slot
    # multisets covering `needs` (ordered desc). DFS over waste-ordered options.
    from itertools import product as iproduct

    sizes = {}
    for s in pat:
        if s > 0:
            sizes[s] = sizes.get(s, 0) + NCORES
    svals = sorted(sizes, reverse=True)
    scnt = [sizes[s] for s in svals]
    budget = [0]

    def dfs(i, avail):
        budget[0] += 1
        if budget[0] > 20000:
            return None
        if i == len(needs):
            return []
        need = needs[i]
        if sum(a * s for a, s in zip(avail, svals)) < sum(needs[i:]):
            return None
        opts = []
        maxn = [min(a, -(-need // s) if s else 0) for a, s in zip(avail, svals)]
        for combo in iproduct(*[range(m + 1) for m in maxn]):
            cap = sum(n * s for n, s in zip(combo, svals))
            if cap < need:
                continue
            # drop combos with a removable slot
            if any(n > 0 and cap - s >= need for n, s in zip(combo, svals)):
                continue
            opts.append((cap - need, combo))
        opts.sort()
        for _, combo in opts:
            rest = dfs(i + 1, [a - n for a, n in zip(avail, combo)])
            if rest is not None:
                got = []
                for n, s in zip(combo, svals):
                    got += [s] * n
                return [got] + rest
        return None

    return dfs(0, scnt)


def _plan(tile_counts):
    # Find per-core slot pattern (a,b,c) and an assignment of the 8*3 slots to
    # experts so each expert k gets slots with total capacity >= tile_counts[k].
    # Returns (pattern, assign) where assign[k] = list of slot sizes granted.
    total = int(sum(tile_counts))
    t_sorted = sorted(range(K), key=lambda k: -tile_counts[k])
    base = -(-total // NCORES)
    best = None
    for t_pc in range(base, base + 3):
        pats = []
        for a in range(-(-t_pc // 3), t_pc + 1):
            for b in range(0, min(a, t_pc - a) + 1):
                c = t_pc - a - b
                if c <= b and c >= 0:
                    pats.append((a, b, c))
        for pat in pats:
            assign = _solve_assign(pat, [int(tile_counts[k]) for k in t_sorted])
            if assign is not None:
                best = (pat, {k: assign[i] for i, k in enumerate(t_sorted)})
                break
        if best is not None:
            break
    if best is None:
        # fallback: expert-sharded, one slot per core
        t_max = max(1, int(max(tile_counts)))
        return (t_max, 0, 0), {k: [t_max] for k in range(K)}
    return best


def build_nc(pattern):
    # Per-core: slots of `pattern` tiles, each slot has its own folded expert
    # matrices. Per 128-row tile: 5 fp8 DoubleRow matmuls accumulate
    #   e = x0@(4M) + u@(8B) - x1@(4W^T)   (pairwise scales cancel)
    # into one PSUM bank; ACT squares from PSUM, DVE reduces, Pool sums at end.
    slots = [s for s in pattern if s > 0]
    t_pc = sum(slots)
    nc = bacc.Bacc("TRN2", target_bir_lowering=False)
    zt = nc.declare_dram_parameter("zt", [P, t_pc, 2 * NJ, P], FP8, isOutput=False)
    das, dbs = [], []
    for s in range(len(slots)):
        das.append(nc.declare_dram_parameter(f"da{s}", [P, 4, ENC], FP8, isOutput=False))
        dbs.append(nc.declare_dram_parameter(f"db{s}", [P, 6, ENC], FP8, isOutput=False))
    loss = nc.declare_dram_parameter("loss_out", [1, 1], F32, isOutput=True)

    # z DMA groups: small first for early start, then 4-wide
    groups, off = [], 0
    for w in [2, 2]:
        if off < t_pc:
            w = min(w, t_pc - off)
            groups.append((off, w))
            off += w
    while off < t_pc:
        w = min(4, t_pc - off)
        groups.append((off, w))
        off += w
    t2g = {}
    for gi, (goff, w) in enumerate(groups):
        for t in range(goff, goff + w):
            t2g[t] = gi
    # slot of each tile
    t2s, soff = {}, [0]
    for si, s in enumerate(slots):
        for t in range(soff[-1], soff[-1] + s):
            t2s[t] = si
        soff.append(soff[-1] + s)

    with tile.TileContext(nc) as tc, ExitStack() as ctx:
        const = ctx.enter_context(tc.tile_pool(name="const", bufs=1))
        stream = ctx.enter_context(tc.tile_pool(name="stream", bufs=len(groups)))
        dwork = ctx.enter_context(tc.tile_pool(name="dwork", bufs=4))
        psum = ctx.enter_context(tc.tile_pool(name="psum", bufs=8, space="PSUM"))

        # PE warmup on zeroed scratch: ramps the p-state while DMAs run.
        wz = const.tile([P, 2, P], FP8)
        wd = const.tile([P, 2, ENC], FP8)
        nc.gpsimd.memset(wz[:], 0)
        nc.gpsimd.memset(wd[:], 0)
        for _ in range(NWARM):
            pw = psum.tile([P, ENC], F32, name="pw", tag="ps")
            nc.tensor.matmul(pw[:], wz[:], wd[:], start=True, stop=True, perf_mode=DR)

        da_sb = [const.tile([P, 4, ENC], FP8, name=f"dasb{s}") for s in range(len(slots))]
        db_sb = [const.tile([P, 6, ENC], FP8, name=f"dbsb{s}") for s in range(len(slots))]
        acc = const.tile([P, t_pc], F32)

        def d_ap(si, j):
            return (
                da_sb[si][:, 2 * j : 2 * j + 2, :]
                if j < 2
                else db_sb[si][:, 2 * (j - 2) : 2 * (j - 2) + 2, :]
            )

        # DMA issue order: slot-0 matrices, early z groups, then interleave
        # remaining matrices ahead of the z tiles that need them.
        issued_d = set()
        zs = [None] * len(groups)

        def issue_d(si):
            if si not in issued_d:
                issued_d.add(si)
                nc.sync.dma_start(da_sb[si][:], das[si][:])
                nc.sync.dma_start(db_sb[si][:], dbs[si][:])

        issue_d(0)
        for gi, (goff, w) in enumerate(groups):
            # make sure matrices for slots touched by the NEXT group are in flight
            zg = stream.tile([P, 4, 2 * NJ, P], FP8, name="zg")
            nc.sync.dma_start(zg[:, 0:w], zt[:, goff : goff + w])
            zs[gi] = zg
            if gi + 1 < len(groups):
                ngoff, nw = groups[gi + 1]
                for t in range(ngoff, ngoff + nw):
                    issue_d(t2s[t])

        for t in range(t_pc):
            gi = t2g[t]
            i = t - groups[gi][0]
            si = t2s[t]
            ps = psum.tile([P, ENC], F32, name="ps", tag="ps")
            for j in range(NJ):
                nc.tensor.matmul(
                    ps[:],
                    zs[gi][:, i, 2 * j : 2 * j + 2, :],
                    d_ap(si, j),
                    start=(j == 0),
                    stop=(j == NJ - 1),
                    perf_mode=DR,
                )
            sj = dwork.tile([P, ENC], BF16, name="sj")
            if t >= t_pc - 2:
                # tail-critical tiles: single ACT with accumulate, no DVE hop
                nc.scalar.activation(
                    sj[:],
                    ps[:],
                    mybir.ActivationFunctionType.Square,
                    accum_out=acc[:, t : t + 1],
                )
            else:
                nc.scalar.activation(sj[:], ps[:], mybir.ActivationFunctionType.Square)
                nc.vector.tensor_reduce(
                    acc[:, t : t + 1],
                    sj[:],
                    axis=mybir.AxisListType.X,
                    op=mybir.AluOpType.add,
                )

        # overlap the bulk of the final reduction with the last tiles
        out_a = const.tile([1, 1], F32)
        out_b = const.tile([1, 1], F32)
        out_sb = const.tile([1, 1], F32)
        nc.gpsimd.tensor_reduce(
            out_a[:],
            acc[:, 0 : t_pc - 2],
            axis=mybir.AxisListType.XYZWC,
            op=mybir.AluOpType.add,
        )
        nc.gpsimd.tensor_reduce(
            out_b[:],
            acc[:, t_pc - 2 : t_pc],
            axis=mybir.AxisListType.XYZWC,
            op=mybir.AluOpType.add,
        )
        nc.gpsimd.tensor_tensor(out_sb[:], out_a[:], out_b[:], mybir.AluOpType.add)
        nc.sync.dma_start(loss[:], out_sb[:])

    nc.finalize()
    return nc


_NC_CACHE = {}
_LAST_KEY = None


def _get_nc(pattern=None):
    key = _LAST_KEY if pattern is None else pattern
    if key not in _NC_CACHE:
        _NC_CACHE[key] = build_nc(key)
    return _NC_CACHE[key]


def make_in_maps(X1, X0, U, W_enc, A_all, B_rest, C_w, C_b):
    global _LAST_KEY
    X1, X0, U = np.asarray(X1), np.asarray(X0), np.asarray(U)
    W_enc, A_all, B_rest = np.asarray(W_enc), np.asarray(A_all), np.asarray(B_rest)
    C_w, C_b = np.asarray(C_w), np.asarray(C_b)

    # f64 router on host: argmax(X0 @ W_enc.T @ C_w.T + C_b) per row
    m = (C_w.astype(np.float64) @ W_enc.astype(np.float64)).T  # [OBS, K]
    inds = np.argmax(X0.astype(np.float64) @ m + C_b.astype(np.float64), axis=1)
    counts = np.bincount(inds, minlength=K)
    tile_counts = [-(-int(c) // P) for c in counts]
    pattern, assign = _plan(tile_counts)
    _LAST_KEY = pattern
    slots = [s for s in pattern if s > 0]
    nslot = len(slots)
    t_pc = sum(slots)

    # quantize data once (pair scales cancel against the matrices)
    x0q = (X0 * 0.25).astype(NP8)
    x1q = (X1 * 0.25).astype(NP8)
    uq = (U * 0.125).astype(NP8)

    wT = W_enc.T.astype(np.float32)  # [OBS, ENC]
    wn4 = -4.0 * wT
    B0 = np.eye(ENC, dtype=np.float32)[:ACT]
    Ball = np.concatenate([B0[None], B_rest.astype(np.float32)], axis=0)

    d8 = {}
    for c in range(K):
        m4 = 4.0 * (wT @ A_all[c].T.astype(np.float32))  # [OBS, ENC]
        dslab = np.zeros((2 * NJ, P, ENC), np.float32)
        dslab[0:4] = m4.reshape(4, P, ENC)
        dslab[4:8] = wn4.reshape(4, P, ENC)
        dslab[8, :ACT, :] = 8.0 * Ball[c]
        d8[c] = dslab.astype(NP8)
    dzero = np.zeros((2 * NJ, P, ENC), NP8)

    # distribute each expert's slot grants to (core, slot_index) positions:
    # free positions per slot size, one (a,b,c) triple per core
    free = {si: list(range(NCORES)) for si in range(nslot)}
    # map slot size -> slot indices having that size (sizes can repeat)
    size2si = {}
    for si, s in enumerate(slots):
        size2si.setdefault(s, []).append(si)
    core_slots = [[None] * nslot for _ in range(NCORES)]  # (expert, n_tiles_here)
    for k in sorted(range(K), key=lambda k: -tile_counts[k]):
        rem = tile_counts[k]
        for s in sorted(assign[k], reverse=True):
            placed = False
            for si in size2si[s]:
                if free[si]:
                    c = free[si].pop(0)
                    take = min(rem, s)
                    core_slots[c][si] = (k, take)
                    rem -= take
                    placed = True
                    break
            assert placed, "slot placement failed"
    # rows per expert, consumed in order
    rowptr = {k: 0 for k in range(K)}
    rowlist = {k: np.nonzero(inds == k)[0] for k in range(K)}

    in_maps = []
    soff = np.cumsum([0] + slots)
    for c in range(NCORES):
        zz = np.zeros((2 * NJ, P, t_pc * P), NP8)  # [slab, comp, n]
        im = {}
        for si in range(nslot):
            ent = core_slots[c][si]
            if ent is None:
                im[f"da{si}"] = np.ascontiguousarray(dzero[0:4].transpose(1, 0, 2))
                im[f"db{si}"] = np.ascontiguousarray(dzero[4:10].transpose(1, 0, 2))
                continue
            k, ntile_k = ent
            p0 = rowptr[k]
            rows = rowlist[k][p0 : p0 + ntile_k * P]
            rowptr[k] = p0 + len(rows)
            nr = len(rows)
            n0 = int(soff[si]) * P
            zz[0:4, :, n0 : n0 + nr] = x0q[rows].T.reshape(4, P, nr)
            zz[4:8, :, n0 : n0 + nr] = x1q[rows].T.reshape(4, P, nr)
            zz[8, :ACT, n0 : n0 + nr] = uq[rows].T
            im[f"da{si}"] = np.ascontiguousarray(d8[k][0:4].transpose(1, 0, 2))
            im[f"db{si}"] = np.ascontiguousarray(d8[k][4:10].transpose(1, 0, 2))
        im["zt"] = np.ascontiguousarray(
            zz.reshape(2 * NJ, P, t_pc, P).transpose(1, 2, 0, 3)
        )  # [p, t, slab, r]
        in_maps.append(im)
    return in_maps


def kernel(X1, X0, U, W_enc, A_all, B_rest, C_w, C_b):
    in_maps = make_in_maps(X1, X0, U, W_enc, A_all, B_rest, C_w, C_b)
    nc = _get_nc()
    res = bass_utils.run_bass_kernel_spmd(nc, in_maps, list(range(NCORES)))
    total = sum(float(r["loss_out"][0, 0]) for r in res.results)
    return np.float32(ALPHA * total / (ENC * N))


# revision 17
# speedup vs baseline: 2.4831x; 1.0358x over previous
import sys

sys.path.insert(0, "/opt/trn_rl_repo")

from contextlib import ExitStack

import ml_dtypes
import numpy as np

import concourse.bass as bass
import concourse.mybir as mybir
import concourse.tile as tile
from concourse import bacc, bass_utils

N, OBS, ENC, ACT, K = 16384, 512, 512, 64, 8
ALPHA = 1.0
NCORES = 8
P = 128
NJ = 5  # DoubleRow slab pairs: (x0a,x0b),(x0c,x0d),(x1a,x1b),(x1c,x1d),(u,0)
F32 = mybir.dt.float32
BF16 = mybir.dt.bfloat16
FP8 = mybir.dt.float8e4
DR = mybir.MatmulPerfMode.DoubleRow
NP8 = ml_dtypes.float8_e4m3
NWARM = 14


def _solve_assign(pat, needs):
    # slots: NCORES copies of each nonzero size in pat; find per-expert slot
    # multisets covering `needs` (ordered desc). DFS over waste-ordered options.
    from itertools import product as iproduct

    sizes = {}
    for s in pat:
        if s > 0:
            sizes[s] = sizes.get(s, 0) + NCORES
    svals = sorted(sizes, reverse=True)
    scnt = [sizes[s] for s in svals]
    budget = [0]

    def dfs(i, avail):
        budget[0] += 1
        if budget[0] > 20000:
            return None
        if i == len(needs):
            return []
        need = needs[i]
        if sum(a * s for a, s in zip(avail, svals)) < sum(needs[i:]):
            return None
        opts = []
        maxn = [min(a, -(-need // s) if s else 0) for a, s in zip(avail, svals)]
        for combo in iproduct(*[range(m + 1) for m in maxn]):
            cap = sum(n * s for n, s in zip(combo, svals))
            if cap < need:
                continue
            # drop combos with a removable slot
            if any(n > 0 and cap - s >= need for n, s in zip(combo, svals)):
                continue
            opts.append((cap - need, combo))
        opts.sort()
        for _, combo in opts:
            rest = dfs(i + 1, [a - n for a, n in zip(avail, combo)])
            if rest is not None:
                got = []
                for n, s in zip(combo, svals):
                    got += [s] * n
                return [got] + rest
        return None

    return dfs(0, scnt)


def _plan(tile_counts):
    # Find per-core slot pattern (a,b,c) and an assignment of the 8*3 slots to
    # experts so each expert k gets slots with total capacity >= tile_counts[k].
    # Returns (pattern, assign) where assign[k] = list of slot sizes granted.
    total = int(sum(tile_counts))
    t_sorted = sorted(range(K), key=lambda k: -tile_counts[k])
    base = -(-total // NCORES)
    best = None
    for t_pc in range(base, base + 3):
        pats = []
        for a in range(-(-t_pc // 3), t_pc + 1):
            for b in range(0, min(a, t_pc - a) + 1):
                c = t_pc - a - b
                if c <= b and c >= 0:
                    pats.append((a, b, c))
        for pat in pats:
            assign = _solve_assign(pat, [int(tile_counts[k]) for k in t_sorted])
            if assign is not None:
                best = (pat, {k: assign[i] for i, k in enumerate(t_sorted)})
                break
        if best is not None:
            break
    if best is None:
        # fallback: expert-sharded, one slot per core
        t_max = max(1, int(max(tile_counts)))
        return (t_max, 0, 0), {k: [t_max] for k in range(K)}
    return best


def build_nc(pattern):
    # Per-core: slots of `pattern` tiles, each slot has its own folded expert
    # matrices. Per 128-row tile: 5 fp8 DoubleRow matmuls accumulate
    #   e = x0@(4M) + u@(8B) - x1@(4W^T)   (pairwise scales cancel)
    # into one PSUM bank; ACT squares from PSUM, DVE reduces, Pool sums at end.
    slots = [s for s in pattern if s > 0]
    t_pc = sum(slots)
    nc = bacc.Bacc("TRN2", target_bir_lowering=False)
    zt = nc.declare_dram_parameter("zt", [P, t_pc, 2 * NJ, P], FP8, isOutput=False)
    das, dbs = [], []
    for s in range(len(slots)):
        das.append(nc.declare_dram_parameter(f"da{s}", [P, 4, ENC], FP8, isOutput=False))
        dbs.append(nc.declare_dram_parameter(f"db{s}", [P, 6, ENC], FP8, isOutput=False))
    loss = nc.declare_dram_parameter("loss_out", [1, 1], F32, isOutput=True)

    # z DMA groups: small first for early start, then 4-wide
    groups, off = [], 0
    for w in [1, 1, 2, 3]:
        if off < t_pc:
            w = min(w, t_pc - off)
            groups.append((off, w))
            off += w
    while off < t_pc:
        w = min(4, t_pc - off)
        groups.append((off, w))
        off += w
    t2g = {}
    for gi, (goff, w) in enumerate(groups):
        for t in range(goff, goff + w):
            t2g[t] = gi
    # slot of each tile
    t2s, soff = {}, [0]
    for si, s in enumerate(slots):
        for t in range(soff[-1], soff[-1] + s):
            t2s[t] = si
        soff.append(soff[-1] + s)

    with tile.TileContext(nc) as tc, ExitStack() as ctx:
        const = ctx.enter_context(tc.tile_pool(name="const", bufs=1))
        stream = ctx.enter_context(tc.tile_pool(name="stream", bufs=len(groups)))
        dwork = ctx.enter_context(tc.tile_pool(name="dwork", bufs=4))
        psum = ctx.enter_context(tc.tile_pool(name="psum", bufs=8, space="PSUM"))

        # PE warmup on zeroed scratch: ramps the p-state while DMAs run.
        wz = const.tile([P, 2, P], FP8)
        wd = const.tile([P, 2, ENC], FP8)
        nc.gpsimd.memset(wz[:], 0)
        nc.gpsimd.memset(wd[:], 0)
        for _ in range(NWARM):
            pw = psum.tile([P, ENC], F32, name="pw", tag="ps")
            nc.tensor.matmul(pw[:], wz[:], wd[:], start=True, stop=True, perf_mode=DR)

        da_sb = [const.tile([P, 4, ENC], FP8, name=f"dasb{s}") for s in range(len(slots))]
        db_sb = [const.tile([P, 6, ENC], FP8, name=f"dbsb{s}") for s in range(len(slots))]
        acc = const.tile([P, t_pc], F32)

        def d_ap(si, j):
            return (
                da_sb[si][:, 2 * j : 2 * j + 2, :]
                if j < 2
                else db_sb[si][:, 2 * (j - 2) : 2 * (j - 2) + 2, :]
            )

        # DMA issue order: slot-0 matrices, early z groups, then interleave
        # remaining matrices ahead of the z tiles that need them.
        issued_d = set()
        zs = [None] * len(groups)

        def issue_d(si):
            if si not in issued_d:
                issued_d.add(si)
                nc.sync.dma_start(da_sb[si][:], das[si][:])
                nc.sync.dma_start(db_sb[si][:], dbs[si][:])

        nc.sync.dma_start(da_sb[0][:], das[0][:])
        for gi, (goff, w) in enumerate(groups):
            # make sure matrices for slots touched by the NEXT group are in flight
            zg = stream.tile([P, 4, 2 * NJ, P], FP8, name="zg")
            nc.sync.dma_start(zg[:, 0:w], zt[:, goff : goff + w])
            zs[gi] = zg
            if gi == 0:
                nc.sync.dma_start(db_sb[0][:], dbs[0][:])
                issued_d.add(0)
            if gi + 1 < len(groups):
                ngoff, nw = groups[gi + 1]
                for t in range(ngoff, ngoff + nw):
                    issue_d(t2s[t])

        for t in range(t_pc):
            gi = t2g[t]
            i = t - groups[gi][0]
            si = t2s[t]
            ps = psum.tile([P, ENC], F32, name="ps", tag="ps")
            for j in range(NJ):
                nc.tensor.matmul(
                    ps[:],
                    zs[gi][:, i, 2 * j : 2 * j + 2, :],
                    d_ap(si, j),
                    start=(j == 0),
                    stop=(j == NJ - 1),
                    perf_mode=DR,
                )
            sj = dwork.tile([P, ENC], BF16, name="sj")
            nc.scalar.activation(
                sj[:],
                ps[:],
                mybir.ActivationFunctionType.Square,
                accum_out=acc[:, t : t + 1],
            )

        # overlap the bulk of the final reduction with the last tiles
        out_a = const.tile([1, 1], F32)
        out_b = const.tile([1, 1], F32)
        out_sb = const.tile([1, 1], F32)
        nc.gpsimd.tensor_reduce(
            out_a[:],
            acc[:, 0 : t_pc - 2],
            axis=mybir.AxisListType.XYZWC,
            op=mybir.AluOpType.add,
        )
        nc.gpsimd.tensor_reduce(
            out_b[:],
            acc[:, t_pc - 2 : t_pc],
            axis=mybir.AxisListType.XYZWC,
            op=mybir.AluOpType.add,
        )
        nc.gpsimd.tensor_tensor(out_sb[:], out_a[:], out_b[:], mybir.AluOpType.add)
        nc.sync.dma_start(loss[:], out_sb[:])

    nc.finalize()
    return nc


_NC_CACHE = {}
_LAST_KEY = None


def _get_nc(pattern=None):
    key = _LAST_KEY if pattern is None else pattern
    if key not in _NC_CACHE:
        _NC_CACHE[key] = build_nc(key)
    return _NC_CACHE[key]


def make_in_maps(X1, X0, U, W_enc, A_all, B_rest, C_w, C_b):
    global _LAST_KEY
    X1, X0, U = np.asarray(X1), np.asarray(X0), np.asarray(U)
    W_enc, A_all, B_rest = np.asarray(W_enc), np.asarray(A_all), np.asarray(B_rest)
    C_w, C_b = np.asarray(C_w), np.asarray(C_b)

    # f64 router on host: argmax(X0 @ W_enc.T @ C_w.T + C_b) per row
    m = (C_w.astype(np.float64) @ W_enc.astype(np.float64)).T  # [OBS, K]
    inds = np.argmax(X0.astype(np.float64) @ m + C_b.astype(np.float64), axis=1)
    counts = np.bincount(inds, minlength=K)
    tile_counts = [-(-int(c) // P) for c in counts]
    pattern, assign = _plan(tile_counts)
    _LAST_KEY = pattern
    slots = [s for s in pattern if s > 0]
    nslot = len(slots)
    t_pc = sum(slots)

    # quantize data once (pair scales cancel against the matrices)
    x0q = (X0 * 0.25).astype(NP8)
    x1q = (X1 * 0.25).astype(NP8)
    uq = (U * 0.125).astype(NP8)

    wT = W_enc.T.astype(np.float32)  # [OBS, ENC]
    wn4 = -4.0 * wT
    B0 = np.eye(ENC, dtype=np.float32)[:ACT]
    Ball = np.concatenate([B0[None], B_rest.astype(np.float32)], axis=0)

    d8 = {}
    for c in range(K):
        m4 = 4.0 * (wT @ A_all[c].T.astype(np.float32))  # [OBS, ENC]
        dslab = np.zeros((2 * NJ, P, ENC), np.float32)
        dslab[0:4] = m4.reshape(4, P, ENC)
        dslab[4:8] = wn4.reshape(4, P, ENC)
        dslab[8, :ACT, :] = 8.0 * Ball[c]
        d8[c] = dslab.astype(NP8)
    dzero = np.zeros((2 * NJ, P, ENC), NP8)

    # distribute each expert's slot grants to (core, slot_index) positions:
    # free positions per slot size, one (a,b,c) triple per core
    free = {si: list(range(NCORES)) for si in range(nslot)}
    # map slot size -> slot indices having that size (sizes can repeat)
    size2si = {}
    for si, s in enumerate(slots):
        size2si.setdefault(s, []).append(si)
    core_slots = [[None] * nslot for _ in range(NCORES)]  # (expert, n_tiles_here)
    for k in sorted(range(K), key=lambda k: -tile_counts[k]):
        rem = tile_counts[k]
        for s in sorted(assign[k], reverse=True):
            placed = False
            for si in size2si[s]:
                if free[si]:
                    c = free[si].pop(0)
                    take = min(rem, s)
                    core_slots[c][si] = (k, take)
                    rem -= take
                    placed = True
                    break
            assert placed, "slot placement failed"
    # rows per expert, consumed in order
    rowptr = {k: 0 for k in range(K)}
    rowlist = {k: np.nonzero(inds == k)[0] for k in range(K)}

    in_maps = []
    soff = np.cumsum([0] + slots)
    for c in range(NCORES):
        zz = np.zeros((2 * NJ, P, t_pc * P), NP8)  # [slab, comp, n]
        im = {}
        for si in range(nslot):
            ent = core_slots[c][si]
            if ent is None:
                im[f"da{si}"] = np.ascontiguousarray(dzero[0:4].transpose(1, 0, 2))
                im[f"db{si}"] = np.ascontiguousarray(dzero[4:10].transpose(1, 0, 2))
                continue
            k, ntile_k = ent
            p0 = rowptr[k]
            rows = rowlist[k][p0 : p0 + ntile_k * P]
            rowptr[k] = p0 + len(rows)
            nr = len(rows)
            n0 = int(soff[si]) * P
            zz[0:4, :, n0 : n0 + nr] = x0q[rows].T.reshape(4, P, nr)
            zz[4:8, :, n0 : n0 + nr] = x1q[rows].T.reshape(4, P, nr)
            zz[8, :ACT, n0 : n0 + nr] = uq[rows].T
            im[f"da{si}"] = np.ascontiguousarray(d8[k][0:4].transpose(1, 0, 2))
            im[f"db{si}"] = np.ascontiguousarray(d8[k][4:10].transpose(1, 0, 2))
        im["zt"] = np.ascontiguousarray(
            zz.reshape(2 * NJ, P, t_pc, P).transpose(1, 2, 0, 3)
        )  # [p, t, slab, r]
        in_maps.append(im)
    return in_maps


def kernel(X1, X0, U, W_enc, A_all, B_rest, C_w, C_b):
    in_maps = make_in_maps(X1, X0, U, W_enc, A_all, B_rest, C_w, C_b)
    nc = _get_nc()
    res = bass_utils.run_bass_kernel_spmd(nc, in_maps, list(range(NCORES)))
    total = sum(float(r["loss_out"][0, 0]) for r in res.results)
    return np.float32(ALPHA * total / (ENC * N))
